# revision 2
# baseline (speedup 1.0000x reference)
"""STFT (n_fft=4096, hop=1024, centered reflect-pad, Hann) on 8 TRN2 cores.

Algorithm: 2-stage Cooley-Tukey, n = 128*n1 + n2 (n1 in [0,32), n2 in [0,128)),
k = k1 + 32*k2 (k1 in [0,32), k2 in [0,64] for the 2049 kept bins).

  X[k1+32k2, b] = sum_n2 G[n2,k] * sum_n1 e^{-2pi i n1 k1/32} * xw[b, 128n1+n2]

Stage 1 runs frames-as-weights so its output lands transposed (n2 on
partitions): per 4-frame subgroup one [128,128] lhsT (4 frames interleaved
across partitions) against a constant one-hot-structured rhs [128,256].
Stage 2 contracts n2 (K=128) with per-k1 twiddle matrices in fp16.

Windowing is folded into host-side input prep: 4 phase-shifted windowed
copies of the padded signal (xw_j = xp * w[1024j + p mod 1024]); the framing
DMA reads each frame quarter from the matching copy.

Partition mapping of stage-1 lhsT rows: p = 32*j + 4*i + r  (j = quarter,
i = n1 mod 8 ... n1 = 8j+i, r = frame-in-subgroup). Each (plane, j) framing
DMA then fills 32 contiguous partitions in one shot.

Sharding: frame-parallel. Core i computes 516 frames starting at frame 512*i
(SPMD, same NEFF); host trims/concatenates to the 4097 global frames.
"""

import numpy as np

import concourse.bacc as bacc
import concourse.tile as tile
import concourse.mybir as mybir
from concourse import bass_utils

N_FFT = 4096
HOP = 1024
T = 4194304
NBINS = N_FFT // 2 + 1          # 2049
F_TOTAL = T // HOP + 1          # 4097
NCORES = 8

NF = 516                        # frames computed per core (129 subgroups of 4)
GROUPS = [128, 128, 128, 128, 4]
L = (NF - 1) * HOP + N_FFT      # per-core input samples per plane = 531456
XW_LEN = 2 * L + 8192           # flat [plane0 | plane1 | slack] per xw tensor

F32R = mybir.dt.float32r
F32 = mybir.dt.float32
F16 = mybir.dt.float16

_cache = {}


def _host_constants():
    n1 = np.arange(32)
    k1 = np.arange(32)
    C = np.cos(2 * np.pi * np.outer(n1, k1) / 32)
    S = np.sin(2 * np.pi * np.outer(n1, k1) / 32)
    R1 = np.concatenate([C, -S], axis=1)      # [n1, 64]
    R2 = np.concatenate([S, C], axis=1)
    # lhsT partition p = 32j + 8r + i  <->  (n1 = 8j+i, frame r)
    R1D = np.zeros((128, 256), np.float32)
    R2D = np.zeros((128, 256), np.float32)
    for j in range(4):
        for i in range(8):
            for r in range(4):
                p = 32 * j + 8 * r + i
                R1D[p, 64 * r:64 * r + 64] = R1[8 * j + i]
                R2D[p, 64 * r:64 * r + 64] = R2[8 * j + i]

    n2 = np.arange(128)
    k2 = np.arange(64)
    Gp = np.zeros((128, 32 * 128), np.float16)
    Gq = np.zeros((128, 32 * 128), np.float16)
    for q in range(32):
        kk = q + 32 * k2
        ang = 2 * np.pi * np.outer(n2, kk) / N_FFT
        gr = np.cos(ang)
        gi = -np.sin(ang)
        Gp[:, 128 * q:128 * q + 64] = gr.astype(np.float16)
        Gp[:, 128 * q + 64:128 * q + 128] = gi.astype(np.float16)
        Gq[:, 128 * q:128 * q + 64] = (-gi).astype(np.float16)
        Gq[:, 128 * q + 64:128 * q + 128] = gr.astype(np.float16)

    alt = ((-1.0) ** n2).astype(np.float16)
    E1 = np.zeros((128, 2), np.float16)
    E2 = np.zeros((128, 2), np.float16)
    E1[:, 0] = alt
    E2[:, 1] = alt
    return (R1D, R2D, Gp, Gq, E1, E2)


def _build(stages=("dma", "s1", "s2", "out")):
    stages = set(stages)
    nc = bacc.Bacc("TRN2", target_bir_lowering=False, debug=False,
                   enable_asserts=False, num_devices=NCORES)
    xw = [nc.dram_tensor(f"xw{j}", [XW_LEN], F32R, kind="ExternalInput")
          for j in range(4)]
    r1d = nc.dram_tensor("r1d", [128, 256], F32R, kind="ExternalInput")
    r2d = nc.dram_tensor("r2d", [128, 256], F32R, kind="ExternalInput")
    gp = nc.dram_tensor("gp", [128, 32 * 128], F16, kind="ExternalInput")
    gq = nc.dram_tensor("gq", [128, 32 * 128], F16, kind="ExternalInput")
    e1 = nc.dram_tensor("e1", [128, 2], F16, kind="ExternalInput")
    e2 = nc.dram_tensor("e2", [128, 2], F16, kind="ExternalInput")
    out = nc.dram_tensor("o", [2, 2048, NF], F32, kind="ExternalOutput")
    oute = nc.dram_tensor("oe", [2, 1, NF], F32, kind="ExternalOutput")

    with tile.TileContext(nc) as tc:
        with (
            tc.tile_pool(name="const", bufs=1) as cpool,
            tc.tile_pool(name="fr", bufs=2) as frpool,
            tc.tile_pool(name="ys", bufs=2) as yspool,
            tc.tile_pool(name="ost", bufs=2) as ostpool,
            tc.tile_pool(name="ps1", bufs=3, space="PSUM") as ps1pool,
            tc.tile_pool(name="ps2", bufs=3, space="PSUM") as ps2pool,
            tc.tile_pool(name="pse", bufs=2, space="PSUM") as psepool,
        ):
            t_r1 = cpool.tile([128, 256], F32R, tag="r1")
            t_r2 = cpool.tile([128, 256], F32R, tag="r2")
            t_gp = cpool.tile([128, 32 * 128], F16, tag="gp")
            t_gq = cpool.tile([128, 32 * 128], F16, tag="gq")
            t_e1 = cpool.tile([128, 2], F16, tag="e1")
            t_e2 = cpool.tile([128, 2], F16, tag="e2")
            nc.sync.dma_start(t_r1[:], r1d.ap()[:, :])
            nc.sync.dma_start(t_r2[:], r2d.ap()[:, :])
            nc.sync.dma_start(t_gp[:], gp.ap()[:, :])
            nc.sync.dma_start(t_gq[:], gq.ap()[:, :])
            nc.sync.dma_start(t_e1[:], e1.ap()[:, :])
            nc.sync.dma_start(t_e2[:], e2.ap()[:, :])

            gb0 = 0
            group_state = []          # (gb0, B, ys) awaiting stage-2
            starts = []
            for B in GROUPS:
                starts.append(gb0)
                gb0 += B

            def emit_load_s1(gb0, B):
                nsub = B // 4
                ncols = 128 * nsub
                fr_r = frpool.tile([128, 128 * 32], F32R, tag="fr_r")
                fr_i = frpool.tile([128, 128 * 32], F32R, tag="fr_i")
                # framing DMA: FR[32j+8r+i, 128s+m] =
                #   xw_j[c*L + 1024*(gb0+4s+r) + 1024j + 128i + m]
                # = dense read of xw_j[off0 : off0+4096*nsub] as (s p m)
                for c, fr_t in ((0, fr_r), (1, fr_i)):
                    for j in range(4):
                        off0 = c * L + HOP * gb0 + 1024 * j
                        seg = xw[j].ap()[off0:off0 + 4096 * nsub]
                        srca = seg.rearrange("(s p m) -> p s m", p=32, m=128)
                        dst = fr_t[32 * j:32 * j + 32, 0:ncols]
                        dst = dst.rearrange("p (s m) -> p s m", m=128)
                        nc.sync.dma_start(dst, srca)

                ys = yspool.tile([128, 64 * 128], F16, tag="ys")
                if "s1" not in stages:
                    return ys
                npair = (nsub + 1) // 2
                for sp in range(npair):
                    s0 = 2 * sp
                    nsg = min(2, nsub - s0)
                    w = 256 * nsg
                    ps1 = ps1pool.tile([128, 512], F32, tag="ps1")
                    for t in range(nsg):
                        s = s0 + t
                        cs = 256 * t
                        nc.tensor.matmul(ps1[:, cs:cs + 256],
                                         fr_r[:, 128 * s:128 * s + 128],
                                         t_r1[:], start=(t == 0), stop=False)
                        nc.tensor.matmul(ps1[:, cs:cs + 256],
                                         fr_i[:, 128 * s:128 * s + 128],
                                         t_r2[:], start=False,
                                         stop=(t == nsg - 1))
                    dstc = ys[:, 256 * s0:256 * s0 + w]
                    if sp % 2 == 0:
                        nc.vector.tensor_copy(dstc, ps1[:, 0:w])
                    else:
                        nc.scalar.copy(dstc, ps1[:, 0:w])
                return ys

            def emit_s2_out(gb0, B, ys):
                if "s2" not in stages:
                    return
                ysv = ys[:, 0:64 * B].rearrange("p (b j) -> p j b", j=64)
                ost = ostpool.tile([128, 32 * 128], F32, tag="ost")
                for qp in range(16):
                    q0 = 2 * qp
                    ps2 = ps2pool.tile([128, 256], F32, tag="ps2")
                    for t in range(2):
                        q = q0 + t
                        rhs_r = ysv[:, q:q + 1, :].rearrange(
                            "p o b -> p (o b)")
                        rhs_i = ysv[:, 32 + q:33 + q, :].rearrange(
                            "p o b -> p (o b)")
                        cs = 128 * t
                        nc.tensor.matmul(ps2[:, cs:cs + B],
                                         t_gp[:, 128 * q:128 * q + 128],
                                         rhs_r, start=(t == 0), stop=False)
                        nc.tensor.matmul(ps2[:, cs:cs + B],
                                         t_gq[:, 128 * q:128 * q + 128],
                                         rhs_i, start=False, stop=(t == 1))
                    d0 = ost[:, 128 * q0:128 * q0 + B]
                    d1 = ost[:, 128 * (q0 + 1):128 * (q0 + 1) + B]
                    if qp % 2 == 0:
                        nc.vector.tensor_copy(d0, ps2[:, 0:B])
                        nc.vector.tensor_copy(d1, ps2[:, 128:128 + B])
                    else:
                        nc.scalar.copy(d0, ps2[:, 0:B])
                        nc.scalar.copy(d1, ps2[:, 128:128 + B])
                    if "out" in stages and qp == 7:
                        srcp = ost[:, 0:16 * 128].rearrange(
                            "p (q b) -> p q b", b=128)[:, :, 0:B]
                        dst = out.ap()[:, :, gb0:gb0 + B]
                        dst = dst.rearrange(
                            "c (p q) b -> (c p) q b", q=32)[:, 0:16, :]
                        nc.scalar.dma_start(dst, srcp)

                # bin 2048 (k1=0, k2=64)
                pse = psepool.tile([2, 128], F32, tag="pse")
                rhs_r0 = ysv[:, 0:1, :].rearrange("p o b -> p (o b)")
                rhs_i0 = ysv[:, 32:33, :].rearrange("p o b -> p (o b)")
                nc.tensor.matmul(pse[:, 0:B], t_e1[:], rhs_r0,
                                 start=True, stop=False)
                nc.tensor.matmul(pse[:, 0:B], t_e2[:], rhs_i0,
                                 start=False, stop=True)
                oste = ostpool.tile([2, 128], F32, tag="oste")
                nc.vector.tensor_copy(oste[:, 0:B], pse[:, 0:B])

                if "out" in stages:
                    srcp = ost[:, 16 * 128:].rearrange(
                        "p (q b) -> p q b", b=128)[:, :, 0:B]
                    dst = out.ap()[:, :, gb0:gb0 + B]
                    dst = dst.rearrange(
                        "c (p q) b -> (c p) q b", q=32)[:, 16:32, :]
                    nc.scalar.dma_start(dst, srcp)
                    dste = oute.ap()[:, 0, gb0:gb0 + B]
                    nc.scalar.dma_start(dste, oste[:, 0:B])

            pending = None
            for gi, B in enumerate(GROUPS):
                ys = emit_load_s1(starts[gi], B)
                if pending is not None:
                    emit_s2_out(*pending)
                pending = (starts[gi], B, ys)
            emit_s2_out(*pending)

    nc.compile()
    return nc


def _prep_inputs(x, window):
    pad = N_FFT // 2
    xp = np.pad(np.asarray(x), ((0, 0), (pad, pad)), mode="reflect")
    total = xp.shape[1]
    need = (NCORES - 1) * 512 * HOP + L
    xp_ext = np.zeros((2, max(total, need)), np.float32)
    xp_ext[:, :total] = xp
    w = np.asarray(window, np.float32)
    reps = xp_ext.shape[1] // HOP + 1
    xw_full = []
    for j in range(4):
        wj = np.tile(w[HOP * j:HOP * (j + 1)], reps)[:xp_ext.shape[1]]
        xw_full.append(xp_ext * wj[None, :])
    return xw_full


def kernel(x, window):
    import time
    t0 = time.time()
    x = np.asarray(x, np.float32)
    window = np.asarray(window, np.float32)
    if "nc" not in _cache:
        _cache["nc"] = _build()
    nc = _cache["nc"]
    print(f"[kernel] build done {time.time()-t0:.2f}s", flush=True)

    xw_full = _prep_inputs(x, window)
    R1D, R2D, Gp, Gq, E1, E2 = _host_constants()

    in_maps = []
    for i in range(NCORES):
        s0 = i * 512 * HOP
        m = {"r1d": R1D, "r2d": R2D, "gp": Gp, "gq": Gq, "e1": E1, "e2": E2}
        for j in range(4):
            flat = np.zeros(XW_LEN, np.float32)
            flat[:L] = xw_full[j][0, s0:s0 + L]
            flat[L:2 * L] = xw_full[j][1, s0:s0 + L]
            m[f"xw{j}"] = flat
        in_maps.append(m)

    print(f"[kernel] inputs prepped {time.time()-t0:.2f}s", flush=True)
    import os
    kw = {}
    if os.environ.get("BASS_TRACE"):
        kw["tmpdir"] = os.environ.get("BASS_TRACE_DIR") or None
    res = bass_utils.run_bass_kernel_spmd(nc, in_maps,
                                          core_ids=list(range(NCORES)), **kw)
    print(f"[kernel] spmd done {time.time()-t0:.2f}s", flush=True)
    global LAST_EXEC_NS, LAST_TRACE
    LAST_EXEC_NS = res.exec_time_ns
    if res.instructions_and_trace is not None:
        LAST_TRACE = res.instructions_and_trace[1]
        print(f"[kernel] trace: {LAST_TRACE}", flush=True)
    out = np.zeros((2, NBINS, F_TOTAL), np.float32)
    for i in range(NCORES):
        o = res.results[i]["o"]
        oe = res.results[i]["oe"]
        f0 = 512 * i
        nf = 513 if i == NCORES - 1 else 512
        out[:, :2048, f0:f0 + nf] = o[:, :, :nf]
        out[:, 2048, f0:f0 + nf] = oe[:, 0, :nf]
    return out



# revision 5
# speedup vs baseline: 1.5992x; 1.5992x over previous
"""STFT (n_fft=4096, hop=1024, centered reflect-pad, Hann) on 8 TRN2 cores.

2-stage Cooley-Tukey, n = 128*n1 + n2 (n1 in [0,32), n2 in [0,128)),
k = k1 + 32*k2 (k1 in [0,32), k2 in [0,64] for the 2049 kept bins).

  Z[k1+32k2, b] = sum_n2 G[n2,k] * Y[n2,k1,b],
  Y[n2,k1,b]    = sum_n1 e^{-2pi i n1 k1/32} * zw[b, 128 n1 + n2]

Stage 1 packs 2 frames x 2 planes x 32 n1 into the K=128 contraction:
one [128,128] fp16 matmul per 2 frames (lhsT = framed data, rhs = a
constant block-diagonal DFT matrix), output lands with n2 on partitions
as stage 2 needs.  Stage 2 contracts n2 (K=128) with fp16 twiddles,
N=512 frames per matmul.

Host prep materializes the windowed frames directly in the stage-1 lhsT
layout (fp16, dense per-partition runs -> large DMA packets).  Output is
written as one dense fp16 [128, 32*512] tile per core and decoded on
host.  Cores each do 512 frames; the odd 4097th frame is a single
host-side FFT.
"""

import numpy as np

import concourse.bacc as bacc
import concourse.tile as tile
import concourse.mybir as mybir
from concourse import bass_utils

N_FFT = 4096
HOP = 1024
T = 4194304
NBINS = N_FFT // 2 + 1          # 2049
F_TOTAL = T // HOP + 1          # 4097
NCORES = 8

NF = 512                        # frames per core
NS2 = NF // 2                   # 256 two-frame subgroups
CH = 16                         # s2-subgroups per input DMA chunk
NCH = NS2 // CH                 # 16 chunks

F32 = mybir.dt.float32
F16 = mybir.dt.float16

LAST_EXEC_NS = None
LAST_TRACE = None

_cache = {}


def _host_constants():
    n1 = np.arange(32)
    k1 = np.arange(32)
    C = np.cos(2 * np.pi * np.outer(n1, k1) / 32)
    S = np.sin(2 * np.pi * np.outer(n1, k1) / 32)
    R1 = np.concatenate([C, -S], axis=1)      # [n1, 64] -> (Yre | Yim) from xr
    R2 = np.concatenate([S, C], axis=1)       # from xi
    # lhsT partition p = 32j + 16c + 2i + r  (n1 = 8j+i, plane c, frame r)
    R2D = np.zeros((128, 128), np.float16)
    for j in range(4):
        for c in range(2):
            for i in range(8):
                for r in range(2):
                    p = 32 * j + 16 * c + 2 * i + r
                    src = R1 if c == 0 else R2
                    R2D[p, 64 * r:64 * r + 64] = src[8 * j + i]

    n2 = np.arange(128)
    k2 = np.arange(64)
    Gp = np.zeros((128, 32 * 128), np.float16)
    Gq = np.zeros((128, 32 * 128), np.float16)
    for q in range(32):
        kk = q + 32 * k2
        ang = 2 * np.pi * np.outer(n2, kk) / N_FFT
        gr = np.cos(ang)
        gi = -np.sin(ang)
        Gp[:, 128 * q:128 * q + 64] = gr.astype(np.float16)
        Gp[:, 128 * q + 64:128 * q + 128] = gi.astype(np.float16)
        Gq[:, 128 * q:128 * q + 64] = (-gi).astype(np.float16)
        Gq[:, 128 * q + 64:128 * q + 128] = gr.astype(np.float16)

    alt = ((-1.0) ** n2).astype(np.float16)
    E1 = np.zeros((128, 2), np.float16)
    E2 = np.zeros((128, 2), np.float16)
    E1[:, 0] = alt
    E2[:, 1] = alt
    return (R2D, Gp, Gq, E1, E2)


def _build():
    nc = bacc.Bacc("TRN2", target_bir_lowering=False, debug=False,
                   enable_asserts=False, num_devices=NCORES)
    xfr = nc.dram_tensor("xfr", [128, NS2 * 128], F16, kind="ExternalInput")
    r2d = nc.dram_tensor("r2d", [128, 128], F16, kind="ExternalInput")
    gp = nc.dram_tensor("gp", [128, 32 * 128], F16, kind="ExternalInput")
    gq = nc.dram_tensor("gq", [128, 32 * 128], F16, kind="ExternalInput")
    e1 = nc.dram_tensor("e1", [128, 2], F16, kind="ExternalInput")
    e2 = nc.dram_tensor("e2", [128, 2], F16, kind="ExternalInput")
    o2 = nc.dram_tensor("o2", [128, 32 * NF], F16, kind="ExternalOutput")
    o2e = nc.dram_tensor("o2e", [2, NF], F16, kind="ExternalOutput")

    with tile.TileContext(nc) as tc:
        with (
            tc.tile_pool(name="const", bufs=1) as cpool,
            tc.tile_pool(name="fr", bufs=4) as frpool,
            tc.tile_pool(name="ys", bufs=1) as yspool,
            tc.tile_pool(name="ost", bufs=3) as ostpool,
            tc.tile_pool(name="ps1", bufs=4, space="PSUM") as ps1pool,
            tc.tile_pool(name="ps2", bufs=3, space="PSUM") as ps2pool,
            tc.tile_pool(name="pse", bufs=1, space="PSUM") as psepool,
        ):
            t_r2 = cpool.tile([128, 128], F16, tag="r2")
            t_gp = cpool.tile([128, 32 * 128], F16, tag="gp")
            t_gq = cpool.tile([128, 32 * 128], F16, tag="gq")
            t_e1 = cpool.tile([128, 2], F16, tag="e1")
            t_e2 = cpool.tile([128, 2], F16, tag="e2")
            nc.sync.dma_start(t_r2[:], r2d.ap()[:, :])
            nc.sync.dma_start(t_gp[:], gp.ap()[:, :])
            nc.sync.dma_start(t_gq[:], gq.ap()[:, :])
            nc.sync.dma_start(t_e1[:], e1.ap()[:, :])
            nc.sync.dma_start(t_e2[:], e2.ap()[:, :])

            ys = yspool.tile([128, 64 * NF], F16, tag="ys")

            evac = [
                lambda d, s: nc.vector.tensor_copy(d, s),
                lambda d, s: nc.scalar.copy(d, s),
            ]
            nev = len(evac)
            ev = 0

            # ---- stage 1: chunked framing DMA + one matmul per 2 frames
            for ch in range(NCH):
                fr = frpool.tile([128, 128 * CH], F16, tag="fr")
                c0 = 128 * CH * ch
                nc.sync.dma_start(fr[:], xfr.ap()[:, c0:c0 + 128 * CH])
                for t0 in range(0, CH, 4):
                    ps = ps1pool.tile([128, 512], F32, tag="ps1")
                    for u in range(4):
                        nc.tensor.matmul(ps[:, 128 * u:128 * u + 128],
                                         fr[:, 128 * (t0 + u):128 * (t0 + u) + 128],
                                         t_r2[:], start=True, stop=True)
                    dst = ys[:, 128 * (CH * ch + t0):128 * (CH * ch + t0) + 512]
                    evac[ev % nev](dst, ps[:, :])
                    ev += 1

            # ---- stage 2: per k1=q, one N=512 matmul pair
            ysv = ys[:, :].rearrange("p (b j) -> p j b", j=64)
            for q in range(32):
                rhs_r = ysv[:, q:q + 1, :].rearrange("p o b -> p (o b)")
                rhs_i = ysv[:, 32 + q:33 + q, :].rearrange("p o b -> p (o b)")
                ps2 = ps2pool.tile([128, 512], F32, tag="ps2")
                nc.tensor.matmul(ps2[:, :], t_gp[:, 128 * q:128 * q + 128],
                                 rhs_r, start=True, stop=False)
                nc.tensor.matmul(ps2[:, :], t_gq[:, 128 * q:128 * q + 128],
                                 rhs_i, start=False, stop=True)
                if q % 4 == 0:
                    ost = ostpool.tile([128, 4 * NF], F16, tag="ost")
                evac[ev % nev](ost[:, NF * (q % 4):NF * (q % 4) + NF], ps2[:, :])
                ev += 1
                if q % 4 == 3:
                    nc.scalar.dma_start(
                        o2.ap()[:, 4 * NF * (q // 4):4 * NF * (q // 4 + 1)],
                        ost[:, :])

            # ---- bin 2048 (k1=0, k2=64)
            pse = psepool.tile([2, 512], F32, tag="pse")
            rhs_r0 = ysv[:, 0:1, :].rearrange("p o b -> p (o b)")
            rhs_i0 = ysv[:, 32:33, :].rearrange("p o b -> p (o b)")
            nc.tensor.matmul(pse[:, :], t_e1[:], rhs_r0, start=True, stop=False)
            nc.tensor.matmul(pse[:, :], t_e2[:], rhs_i0, start=False, stop=True)
            oste = ostpool.tile([2, 512], F16, tag="oste")
            nc.vector.tensor_copy(oste[:, :], pse[:, :])
            nc.scalar.dma_start(o2e.ap()[:, :], oste[:, :])

    nc.compile()
    return nc


def _prep_inputs(x, window):
    """Windowed frames in the stage-1 lhsT layout, fp16, one array/core."""
    pad = N_FFT // 2
    xp = np.pad(np.asarray(x, np.float32), ((0, 0), (pad, pad)),
                mode="reflect")                       # [2, T + n_fft]
    w = np.asarray(window, np.float32)
    sw = np.lib.stride_tricks.sliding_window_view(xp, N_FFT, axis=1)
    fwin = (sw[:, ::HOP] * w).astype(np.float16)      # [2, 4097, 4096]
    cores = []
    for i in range(NCORES):
        f0 = NF * i
        v = fwin[:, f0:f0 + NF].reshape(2, NS2, 2, 4, 8, 128)
        # [c, s2, r, j, i, m] -> [j, c, i, r][s2, m]
        fr2 = np.ascontiguousarray(
            v.transpose(3, 0, 4, 2, 1, 5)).reshape(128, NS2 * 128)
        cores.append(fr2)
    return cores, xp, w


def kernel(x, window):
    import os
    import time
    t0 = time.time()
    x = np.asarray(x, np.float32)
    window = np.asarray(window, np.float32)
    if "nc" not in _cache:
        _cache["nc"] = _build()
    nc = _cache["nc"]
    print(f"[kernel] build done {time.time()-t0:.2f}s", flush=True)

    cores, xp, w = _prep_inputs(x, window)
    R2D, Gp, Gq, E1, E2 = _host_constants()

    in_maps = []
    for i in range(NCORES):
        in_maps.append({"xfr": cores[i], "r2d": R2D, "gp": Gp, "gq": Gq,
                        "e1": E1, "e2": E2})

    print(f"[kernel] inputs prepped {time.time()-t0:.2f}s", flush=True)
    kw = {}
    if os.environ.get("BASS_TRACE"):
        kw["tmpdir"] = os.environ.get("BASS_TRACE_DIR") or None
    res = bass_utils.run_bass_kernel_spmd(nc, in_maps,
                                          core_ids=list(range(NCORES)), **kw)
    print(f"[kernel] spmd done {time.time()-t0:.2f}s", flush=True)
    global LAST_EXEC_NS, LAST_TRACE
    LAST_EXEC_NS = res.exec_time_ns
    if res.instructions_and_trace is not None:
        LAST_TRACE = res.instructions_and_trace[1]
        print(f"[kernel] trace: {LAST_TRACE}", flush=True)

    out = np.zeros((2, NBINS, F_TOTAL), np.float32)
    for i in range(NCORES):
        f0 = NF * i
        o = res.results[i]["o2"].reshape(2, 64, 32, NF)   # [c, k2, q, b]
        out[:, :2048, f0:f0 + NF] = o.reshape(2, 2048, NF).astype(np.float32)
        out[:, 2048, f0:f0 + NF] = \
            res.results[i]["o2e"].astype(np.float32)

    # the 4097th frame on host (cores each do exactly 512)
    b = F_TOTAL - 1
    seg = xp[:, HOP * b:HOP * b + N_FFT].astype(np.float64)
    Z = np.fft.fft((seg[0] + 1j * seg[1]) * w)
    out[0, :, b] = Z.real[:NBINS].astype(np.float32)
    out[1, :, b] = Z.imag[:NBINS].astype(np.float32)
    return out


# revision 9
# speedup vs baseline: 2.3647x; 1.4787x over previous
"""STFT (n_fft=4096, hop=1024, centered reflect-pad, Hann) on 8 TRN2 cores.

2-stage Cooley-Tukey, n = 128*n1 + n2 (n1 in [0,32), n2 in [0,128)),
k = k1 + 32*k2 (k1 in [0,32), k2 in [0,64] for the 2049 kept bins).

  Z[k1+32k2, b] = sum_n2 G[n2,k] * Y[n2,k1,b],
  Y[n2,k1,b]    = sum_n1 e^{-2pi i n1 k1/32} * zw[b, 128 n1 + n2]

Stage 1 packs 2 frames x 2 planes x 32 n1 into the K=128 contraction:
one [128,128] fp16 matmul per 2 frames (lhsT = framed data, rhs = a
constant block-diagonal DFT matrix), output lands with n2 on partitions
as stage 2 needs.  Stage 2 contracts n2 (K=128) with fp16 twiddles,
N=512 frames per matmul.

Host prep materializes the windowed frames directly in the stage-1 lhsT
layout (fp16, dense per-partition runs -> large DMA packets).  Output is
written as one dense fp16 [128, 32*512] tile per core and decoded on
host.  Cores each do 512 frames; the odd 4097th frame is a single
host-side FFT.
"""

import numpy as np

import concourse.bacc as bacc
import concourse.tile as tile
import concourse.mybir as mybir
from concourse import bass_utils

N_FFT = 4096
HOP = 1024
T = 4194304
NBINS = N_FFT // 2 + 1          # 2049
F_TOTAL = T // HOP + 1          # 4097
NCORES = 8

NF = 512                        # frames per core
NS2 = NF // 2                   # 256 two-frame subgroups
CH = 16                         # s2-subgroups per input DMA chunk
NCH = NS2 // CH                 # 16 chunks

F32 = mybir.dt.float32
F16 = mybir.dt.float16

LAST_EXEC_NS = None
LAST_TRACE = None

_cache = {}


def _host_constants():
    n1 = np.arange(32)
    k1 = np.arange(32)
    C = np.cos(2 * np.pi * np.outer(n1, k1) / 32)
    S = np.sin(2 * np.pi * np.outer(n1, k1) / 32)
    R1 = np.concatenate([C, -S], axis=1)      # [n1, 64] -> (Yre | Yim) from xr
    R2 = np.concatenate([S, C], axis=1)       # from xi
    # lhsT partition p = 32j + 16c + 2i + r  (n1 = 8j+i, plane c, frame r)
    R2D = np.zeros((128, 128), np.float16)
    for j in range(4):
        for c in range(2):
            for i in range(8):
                for r in range(2):
                    p = 32 * j + 16 * c + 2 * i + r
                    src = R1 if c == 0 else R2
                    R2D[p, 64 * r:64 * r + 64] = src[8 * j + i]

    n2 = np.arange(128)
    k2 = np.arange(64)
    Gp = np.zeros((128, 32 * 128), np.float16)
    Gq = np.zeros((128, 32 * 128), np.float16)
    for q in range(32):
        kk = q + 32 * k2
        ang = 2 * np.pi * np.outer(n2, kk) / N_FFT
        gr = np.cos(ang)
        gi = -np.sin(ang)
        Gp[:, 128 * q:128 * q + 64] = gr.astype(np.float16)
        Gp[:, 128 * q + 64:128 * q + 128] = gi.astype(np.float16)
        Gq[:, 128 * q:128 * q + 64] = (-gi).astype(np.float16)
        Gq[:, 128 * q + 64:128 * q + 128] = gr.astype(np.float16)

    alt = ((-1.0) ** n2).astype(np.float16)
    E1 = np.zeros((128, 2), np.float16)
    E2 = np.zeros((128, 2), np.float16)
    E1[:, 0] = alt
    E2[:, 1] = alt
    return (R2D, Gp, Gq, E1, E2)


def _build():
    nc = bacc.Bacc("TRN2", target_bir_lowering=False, debug=False,
                   enable_asserts=False, num_devices=NCORES)
    xfr = nc.dram_tensor("xfr", [128, NS2 * 128], F16, kind="ExternalInput")
    r2d = nc.dram_tensor("r2d", [128, 128], F16, kind="ExternalInput")
    gp = nc.dram_tensor("gp", [128, 32 * 128], F16, kind="ExternalInput")
    gq = nc.dram_tensor("gq", [128, 32 * 128], F16, kind="ExternalInput")
    e1 = nc.dram_tensor("e1", [128, 2], F16, kind="ExternalInput")
    e2 = nc.dram_tensor("e2", [128, 2], F16, kind="ExternalInput")
    o2 = nc.dram_tensor("o2", [128, 32 * NF], F16, kind="ExternalOutput")
    o2e = nc.dram_tensor("o2e", [2, NF], F16, kind="ExternalOutput")

    with tile.TileContext(nc) as tc:
        with (
            tc.tile_pool(name="const", bufs=1) as cpool,
            tc.tile_pool(name="fr", bufs=4) as frpool,
            tc.tile_pool(name="ys", bufs=1) as yspool,
            tc.tile_pool(name="ost", bufs=3) as ostpool,
            tc.tile_pool(name="ps1", bufs=4, space="PSUM") as ps1pool,
            tc.tile_pool(name="ps2", bufs=3, space="PSUM") as ps2pool,
            tc.tile_pool(name="pse", bufs=1, space="PSUM") as psepool,
        ):
            t_r2 = cpool.tile([128, 128], F16, tag="r2")
            t_gp = cpool.tile([128, 32 * 128], F16, tag="gp")
            t_gq = cpool.tile([128, 32 * 128], F16, tag="gq")
            t_e1 = cpool.tile([128, 2], F16, tag="e1")
            t_e2 = cpool.tile([128, 2], F16, tag="e2")
            # r2d (needed first) on the sync ring ahead of the input chunks;
            # stage-2 constants on the scalar ring so they don't delay chunk 0
            nc.sync.dma_start(t_r2[:], r2d.ap()[:, :])
            nc.scalar.dma_start(t_gp[:], gp.ap()[:, :])
            nc.scalar.dma_start(t_gq[:], gq.ap()[:, :])
            nc.scalar.dma_start(t_e1[:], e1.ap()[:, :])
            nc.scalar.dma_start(t_e2[:], e2.ap()[:, :])

            ys = yspool.tile([128, 64 * NF], F16, tag="ys")
            # k1-major view: col = j*NF + b  (contiguous stage-2 rhs)
            ysq = ys[:, :].rearrange("p (j b) -> p j b", b=NF)

            evac = [
                lambda d, s: nc.vector.tensor_copy(d, s),
                lambda d, s: nc.scalar.copy(d, s),
            ]
            nev = len(evac)
            ev = 0

            # ---- stage 1: chunked framing DMA + one matmul per 2 frames
            for ch in range(NCH):
                fr = frpool.tile([128, 128 * CH], F16, tag="fr")
                c0 = 128 * CH * ch
                nc.sync.dma_start(fr[:], xfr.ap()[:, c0:c0 + 128 * CH])
                for t0 in range(0, CH, 4):
                    ps = ps1pool.tile([128, 512], F32, tag="ps1")
                    for u in range(4):
                        nc.tensor.matmul(ps[:, 128 * u:128 * u + 128],
                                         fr[:, 128 * (t0 + u):128 * (t0 + u) + 128],
                                         t_r2[:], start=True, stop=True)
                    # psum col = 128t + 64r + j  ->  ys col = j*NF + b0 + 2t + r
                    b0 = 2 * (CH * ch + t0)
                    src = ps[:, :].rearrange("p (t r j) -> p j (t r)", t=4, r=2)
                    evac[ev % nev](ysq[:, :, b0:b0 + 8], src)
                    ev += 1

            # ---- stage 2: per k1=q, one N=512 matmul pair
            for q in range(32):
                rhs_r = ysq[:, q:q + 1, :].rearrange("p o b -> p (o b)")
                rhs_i = ysq[:, 32 + q:33 + q, :].rearrange("p o b -> p (o b)")
                ps2 = ps2pool.tile([128, 512], F32, tag="ps2")
                nc.tensor.matmul(ps2[:, :], t_gp[:, 128 * q:128 * q + 128],
                                 rhs_r, start=True, stop=False)
                nc.tensor.matmul(ps2[:, :], t_gq[:, 128 * q:128 * q + 128],
                                 rhs_i, start=False, stop=True)
                if q % 4 == 0:
                    ost = ostpool.tile([128, 4 * NF], F16, tag="ost")
                evac[ev % nev](ost[:, NF * (q % 4):NF * (q % 4) + NF], ps2[:, :])
                ev += 1
                if q % 4 == 3:
                    nc.scalar.dma_start(
                        o2.ap()[:, 4 * NF * (q // 4):4 * NF * (q // 4 + 1)],
                        ost[:, :])

            # ---- bin 2048 (k1=0, k2=64)
            pse = psepool.tile([2, 512], F32, tag="pse")
            rhs_r0 = ysq[:, 0:1, :].rearrange("p o b -> p (o b)")
            rhs_i0 = ysq[:, 32:33, :].rearrange("p o b -> p (o b)")
            nc.tensor.matmul(pse[:, :], t_e1[:], rhs_r0, start=True, stop=False)
            nc.tensor.matmul(pse[:, :], t_e2[:], rhs_i0, start=False, stop=True)
            oste = ostpool.tile([2, 512], F16, tag="oste")
            nc.vector.tensor_copy(oste[:, :], pse[:, :])
            nc.scalar.dma_start(o2e.ap()[:, :], oste[:, :])

    nc.compile()
    return nc


def _prep_inputs(x, window):
    """Windowed frames in the stage-1 lhsT layout, fp16, one array/core."""
    pad = N_FFT // 2
    xp = np.pad(np.asarray(x, np.float32), ((0, 0), (pad, pad)),
                mode="reflect")                       # [2, T + n_fft]
    w = np.asarray(window, np.float32)
    sw = np.lib.stride_tricks.sliding_window_view(xp, N_FFT, axis=1)
    fwin = (sw[:, ::HOP] * w).astype(np.float16)      # [2, 4097, 4096]
    cores = []
    for i in range(NCORES):
        f0 = NF * i
        v = fwin[:, f0:f0 + NF].reshape(2, NS2, 2, 4, 8, 128)
        # [c, s2, r, j, i, m] -> [j, c, i, r][s2, m]
        fr2 = np.ascontiguousarray(
            v.transpose(3, 0, 4, 2, 1, 5)).reshape(128, NS2 * 128)
        cores.append(fr2)
    return cores, xp, w


def kernel(x, window):
    import os
    import time
    t0 = time.time()
    x = np.asarray(x, np.float32)
    window = np.asarray(window, np.float32)
    if "nc" not in _cache:
        _cache["nc"] = _build()
    nc = _cache["nc"]
    print(f"[kernel] build done {time.time()-t0:.2f}s", flush=True)

    cores, xp, w = _prep_inputs(x, window)
    R2D, Gp, Gq, E1, E2 = _host_constants()

    in_maps = []
    for i in range(NCORES):
        in_maps.append({"xfr": cores[i], "r2d": R2D, "gp": Gp, "gq": Gq,
                        "e1": E1, "e2": E2})

    print(f"[kernel] inputs prepped {time.time()-t0:.2f}s", flush=True)
    kw = {}
    if os.environ.get("BASS_TRACE"):
        kw["tmpdir"] = os.environ.get("BASS_TRACE_DIR") or None
    res = bass_utils.run_bass_kernel_spmd(nc, in_maps,
                                          core_ids=list(range(NCORES)), **kw)
    print(f"[kernel] spmd done {time.time()-t0:.2f}s", flush=True)
    global LAST_EXEC_NS, LAST_TRACE
    LAST_EXEC_NS = res.exec_time_ns
    if res.instructions_and_trace is not None:
        LAST_TRACE = res.instructions_and_trace[1]
        print(f"[kernel] trace: {LAST_TRACE}", flush=True)

    out = np.zeros((2, NBINS, F_TOTAL), np.float32)
    for i in range(NCORES):
        f0 = NF * i
        o = res.results[i]["o2"].reshape(2, 64, 32, NF)   # [c, k2, q, b]
        out[:, :2048, f0:f0 + NF] = o.reshape(2, 2048, NF).astype(np.float32)
        out[:, 2048, f0:f0 + NF] = \
            res.results[i]["o2e"].astype(np.float32)

    # the 4097th frame on host (cores each do exactly 512)
    b = F_TOTAL - 1
    seg = xp[:, HOP * b:HOP * b + N_FFT].astype(np.float64)
    Z = np.fft.fft((seg[0] + 1j * seg[1]) * w)
    out[0, :, b] = Z.real[:NBINS].astype(np.float32)
    out[1, :, b] = Z.imag[:NBINS].astype(np.float32)
    return out


# revision 13
# speedup vs baseline: 2.5094x; 1.0612x over previous
"""STFT (n_fft=4096, hop=1024, centered reflect-pad, Hann) on 8 TRN2 cores.

2-stage Cooley-Tukey, n = 128*n1 + n2 (n1 in [0,32), n2 in [0,128)),
k = k1 + 32*k2 (k1 in [0,32), k2 in [0,64] for the 2049 kept bins).

  Z[k1+32k2, b] = sum_n2 G[n2,k] * Y[n2,k1,b],
  Y[n2,k1,b]    = sum_n1 e^{-2pi i n1 k1/32} * zw[b, 128 n1 + n2]

Stage 1 packs 2 frames x 2 planes x 32 n1 into the K=128 contraction:
one [128,128] fp16 matmul per 2 frames (lhsT = framed data, rhs = a
constant block-diagonal DFT matrix), output lands with n2 on partitions
as stage 2 needs.  Stage 2 contracts n2 (K=128) with fp16 twiddles,
N=512 frames per matmul.

Host prep materializes the windowed frames directly in the stage-1 lhsT
layout (fp16, dense per-partition runs -> large DMA packets).  Output is
written as one dense fp16 [128, 32*512] tile per core and decoded on
host.  Cores each do 512 frames; the odd 4097th frame is a single
host-side FFT.
"""

import numpy as np

import concourse.bacc as bacc
import concourse.tile as tile
import concourse.mybir as mybir
from concourse import bass_utils

N_FFT = 4096
HOP = 1024
T = 4194304
NBINS = N_FFT // 2 + 1          # 2049
F_TOTAL = T // HOP + 1          # 4097
NCORES = 8

NF = 512                        # frames per core
NS2 = NF // 2                   # 256 two-frame subgroups
CH = 16                         # s2-subgroups per input DMA chunk
NCH = NS2 // CH                 # 16 chunks

F32 = mybir.dt.float32
F16 = mybir.dt.float16

LAST_EXEC_NS = None
LAST_TRACE = None

_cache = {}


def _host_constants():
    n1 = np.arange(32)
    k1 = np.arange(32)
    C = np.cos(2 * np.pi * np.outer(n1, k1) / 32)
    S = np.sin(2 * np.pi * np.outer(n1, k1) / 32)
    R1 = np.concatenate([C, -S], axis=1)      # [n1, 64] -> (Yre | Yim) from xr
    R2 = np.concatenate([S, C], axis=1)       # from xi
    # lhsT partition p = 32j + 16c + 2i + r  (n1 = 8j+i, plane c, frame r)
    R2D = np.zeros((128, 128), np.float16)
    for j in range(4):
        for c in range(2):
            for i in range(8):
                for r in range(2):
                    p = 32 * j + 16 * c + 2 * i + r
                    src = R1 if c == 0 else R2
                    R2D[p, 64 * r:64 * r + 64] = src[8 * j + i]

    n2 = np.arange(128)
    k2 = np.arange(64)
    Gp = np.zeros((128, 32 * 128), np.float16)
    Gq = np.zeros((128, 32 * 128), np.float16)
    for q in range(32):
        kk = q + 32 * k2
        ang = 2 * np.pi * np.outer(n2, kk) / N_FFT
        gr = np.cos(ang)
        gi = -np.sin(ang)
        Gp[:, 128 * q:128 * q + 64] = gr.astype(np.float16)
        Gp[:, 128 * q + 64:128 * q + 128] = gi.astype(np.float16)
        Gq[:, 128 * q:128 * q + 64] = (-gi).astype(np.float16)
        Gq[:, 128 * q + 64:128 * q + 128] = gr.astype(np.float16)

    alt = ((-1.0) ** n2).astype(np.float16)
    E1 = np.zeros((128, 2), np.float16)
    E2 = np.zeros((128, 2), np.float16)
    E1[:, 0] = alt
    E2[:, 1] = alt
    return (R2D, Gp, Gq, E1, E2)


def _build():
    nc = bacc.Bacc("TRN2", target_bir_lowering=False, debug=False,
                   enable_asserts=False, num_devices=NCORES)
    xfr = nc.dram_tensor("xfr", [128, NS2 * 128], F16, kind="ExternalInput")
    r2d = nc.dram_tensor("r2d", [128, 128], F16, kind="ExternalInput")
    gp = nc.dram_tensor("gp", [128, 32 * 128], F16, kind="ExternalInput")
    gq = nc.dram_tensor("gq", [128, 32 * 128], F16, kind="ExternalInput")
    e1 = nc.dram_tensor("e1", [128, 2], F16, kind="ExternalInput")
    e2 = nc.dram_tensor("e2", [128, 2], F16, kind="ExternalInput")
    o2 = nc.dram_tensor("o2", [128, 32 * NF], F16, kind="ExternalOutput")
    o2e = nc.dram_tensor("o2e", [2, NF], F16, kind="ExternalOutput")

    with tile.TileContext(nc) as tc:
        with (
            tc.tile_pool(name="const", bufs=1) as cpool,
            tc.tile_pool(name="fr", bufs=4) as frpool,
            tc.tile_pool(name="ys", bufs=1) as yspool,
            tc.tile_pool(name="ost", bufs=3) as ostpool,
            tc.tile_pool(name="ps1", bufs=3, space="PSUM") as ps1pool,
            tc.tile_pool(name="ps2", bufs=4, space="PSUM") as ps2pool,
            tc.tile_pool(name="pse", bufs=1, space="PSUM") as psepool,
        ):
            t_r2 = cpool.tile([128, 128], F16, tag="r2")
            t_gp = cpool.tile([128, 32 * 128], F16, tag="gp")
            t_gq = cpool.tile([128, 32 * 128], F16, tag="gq")
            t_e1 = cpool.tile([128, 2], F16, tag="e1")
            t_e2 = cpool.tile([128, 2], F16, tag="e2")
            # r2d (needed first) on the sync ring ahead of the input chunks;
            # stage-2 constants are loaded mid-stage-1 (scalar ring) so their
            # HBM traffic doesn't delay chunk 0
            nc.sync.dma_start(t_r2[:], r2d.ap()[:, :])

            ys = yspool.tile([128, 64 * NF], F16, tag="ys")
            # k1-major view: col = j*NF + b  (contiguous stage-2 rhs)
            ysq = ys[:, :].rearrange("p (j b) -> p j b", b=NF)

            evac = [
                lambda d, s: nc.vector.tensor_copy(d, s),
                lambda d, s: nc.scalar.copy(d, s),
            ]
            nev = len(evac)
            ev = 0

            # ---- stage 1: chunked framing DMA + one matmul per 2 frames
            for ch in range(NCH):
                fr = frpool.tile([128, 128 * CH], F16, tag="fr")
                c0 = 128 * CH * ch
                nc.sync.dma_start(fr[:], xfr.ap()[:, c0:c0 + 128 * CH])
                if ch == NCH // 2:
                    nc.scalar.dma_start(t_gp[:], gp.ap()[:, :])
                    nc.scalar.dma_start(t_gq[:], gq.ap()[:, :])
                    nc.scalar.dma_start(t_e1[:], e1.ap()[:, :])
                    nc.scalar.dma_start(t_e2[:], e2.ap()[:, :])
                for t0 in range(0, CH, 4):
                    ps = ps1pool.tile([128, 512], F32, tag="ps1")
                    for u in range(4):
                        nc.tensor.matmul(ps[:, 128 * u:128 * u + 128],
                                         fr[:, 128 * (t0 + u):128 * (t0 + u) + 128],
                                         t_r2[:], start=True, stop=True)
                    # psum col = 128t + 64r + j  ->  ys col = j*NF + b0 + 2t + r
                    b0 = 2 * (CH * ch + t0)
                    src = ps[:, :].rearrange("p (t r j) -> p j (t r)", t=4, r=2)
                    evac[ev % nev](ysq[:, :, b0:b0 + 8], src)
                    ev += 1

            # ---- bin 2048 (k1=0, k2=64) first, so its output DMA overlaps
            pse = psepool.tile([2, 512], F32, tag="pse")
            rhs_r0 = ysq[:, 0:1, :].rearrange("p o b -> p (o b)")
            rhs_i0 = ysq[:, 32:33, :].rearrange("p o b -> p (o b)")
            nc.tensor.matmul(pse[:, :], t_e1[:], rhs_r0, start=True, stop=False)
            nc.tensor.matmul(pse[:, :], t_e2[:], rhs_i0, start=False, stop=True)
            oste = ostpool.tile([2, 512], F16, tag="oste")
            nc.vector.tensor_copy(oste[:, :], pse[:, :])
            nc.scalar.dma_start(o2e.ap()[:, :], oste[:, :])

            # ---- stage 2: per k1=q, one N=512 matmul pair; evacs split
            # across vector+scalar to recycle PSUM banks faster
            for q in range(32):
                rhs_r = ysq[:, q:q + 1, :].rearrange("p o b -> p (o b)")
                rhs_i = ysq[:, 32 + q:33 + q, :].rearrange("p o b -> p (o b)")
                ps2 = ps2pool.tile([128, 512], F32, tag="ps2")
                nc.tensor.matmul(ps2[:, :], t_gp[:, 128 * q:128 * q + 128],
                                 rhs_r, start=True, stop=False)
                nc.tensor.matmul(ps2[:, :], t_gq[:, 128 * q:128 * q + 128],
                                 rhs_i, start=False, stop=True)
                if q % 4 == 0:
                    ost = ostpool.tile([128, 4 * NF], F16, tag="ost")
                o0 = NF * (q % 4)
                nc.vector.tensor_copy(ost[:, o0:o0 + 256], ps2[:, 0:256])
                nc.scalar.copy(ost[:, o0 + 256:o0 + NF], ps2[:, 256:NF])
                if q % 4 == 3:
                    nc.scalar.dma_start(
                        o2.ap()[:, 4 * NF * (q // 4):4 * NF * (q // 4 + 1)],
                        ost[:, :])

    nc.compile()
    return nc


def _prep_inputs(x, window):
    """Windowed frames in the stage-1 lhsT layout, fp16, one array/core."""
    pad = N_FFT // 2
    xp = np.pad(np.asarray(x, np.float32), ((0, 0), (pad, pad)),
                mode="reflect")                       # [2, T + n_fft]
    w = np.asarray(window, np.float32)
    sw = np.lib.stride_tricks.sliding_window_view(xp, N_FFT, axis=1)
    fwin = (sw[:, ::HOP] * w).astype(np.float16)      # [2, 4097, 4096]
    cores = []
    for i in range(NCORES):
        f0 = NF * i
        v = fwin[:, f0:f0 + NF].reshape(2, NS2, 2, 4, 8, 128)
        # [c, s2, r, j, i, m] -> [j, c, i, r][s2, m]
        fr2 = np.ascontiguousarray(
            v.transpose(3, 0, 4, 2, 1, 5)).reshape(128, NS2 * 128)
        cores.append(fr2)
    return cores, xp, w


def kernel(x, window):
    import os
    import time
    t0 = time.time()
    x = np.asarray(x, np.float32)
    window = np.asarray(window, np.float32)
    if "nc" not in _cache:
        _cache["nc"] = _build()
    nc = _cache["nc"]
    print(f"[kernel] build done {time.time()-t0:.2f}s", flush=True)

    cores, xp, w = _prep_inputs(x, window)
    R2D, Gp, Gq, E1, E2 = _host_constants()

    in_maps = []
    for i in range(NCORES):
        in_maps.append({"xfr": cores[i], "r2d": R2D, "gp": Gp, "gq": Gq,
                        "e1": E1, "e2": E2})

    print(f"[kernel] inputs prepped {time.time()-t0:.2f}s", flush=True)
    kw = {}
    if os.environ.get("BASS_TRACE"):
        kw["tmpdir"] = os.environ.get("BASS_TRACE_DIR") or None
    res = bass_utils.run_bass_kernel_spmd(nc, in_maps,
                                          core_ids=list(range(NCORES)), **kw)
    print(f"[kernel] spmd done {time.time()-t0:.2f}s", flush=True)
    global LAST_EXEC_NS, LAST_TRACE
    LAST_EXEC_NS = res.exec_time_ns
    if res.instructions_and_trace is not None:
        LAST_TRACE = res.instructions_and_trace[1]
        print(f"[kernel] trace: {LAST_TRACE}", flush=True)

    out = np.zeros((2, NBINS, F_TOTAL), np.float32)
    for i in range(NCORES):
        f0 = NF * i
        o = res.results[i]["o2"].reshape(2, 64, 32, NF)   # [c, k2, q, b]
        out[:, :2048, f0:f0 + NF] = o.reshape(2, 2048, NF).astype(np.float32)
        out[:, 2048, f0:f0 + NF] = \
            res.results[i]["o2e"].astype(np.float32)

    # the 4097th frame on host (cores each do exactly 512)
    b = F_TOTAL - 1
    seg = xp[:, HOP * b:HOP * b + N_FFT].astype(np.float64)
    Z = np.fft.fft((seg[0] + 1j * seg[1]) * w)
    out[0, :, b] = Z.real[:NBINS].astype(np.float32)
    out[1, :, b] = Z.imag[:NBINS].astype(np.float32)
    return out
